# revision 6
# baseline (speedup 1.0000x reference)
"""Block-sparse attention (CABAttention) Trainium2 kernel, v2.

Sharding: 8 cores = 2 batches x 4 head-groups (4 heads each, as 2 pairs).
Key ideas vs v1:
- All dynamic block selection moved OFF the PE: K blocks gathered with a
  single gpsimd ap_gather per pair (per-16-partition-group index lists
  carry head A vs head B selections); V blocks gathered keys-major with
  indirect DMAs from a DRAM round-trip copy. Every matmul AP is static.
- Softmax denominator comes free from the AV matmul via a ones-column
  appended to the gathered V (no activation accumulator, no reduce).
  Normalization folds into the PSUM->SBUF copy of the AV output.
- fp16 inputs (x^T, qkv w, proj w) halve the input DMA.
- Output projection interleaved into the attention loop (one 128-token
  tile per 4 attention iterations) so its DMA/copies overlap.
Block selection (top-2 + diag) is computed on host in float64 and passed
as gather indices; duplicated-diag slots are masked with a -30000 bias
before exp (baseline-proven).
Unshard: partial output projections summed over the 4 cores per batch,
plus bias.
"""
import sys

sys.path.insert(0, "/opt/trn_rl_repo")

import numpy as np

import concourse.bass as bass
import concourse.mybir as mybir
import concourse.tile as tile
from concourse import bacc
from concourse.bass_utils import run_bass_kernel_spmd
from concourse.masks import make_identity

F32 = mybir.dt.float32
F16 = mybir.dt.float16
I16 = mybir.dt.int16
I32 = mybir.dt.int32

DIM = 1024
H = 16
HD = 64
BS = 64
N = 2048
B = 2
M = N // BS            # 32 blocks
SCALE = HD ** -0.5
NCORES = 8
HPC = H // (NCORES // B)   # 4 heads per core

_NC_CACHE = None
LAST_RESULTS = None


def build_kernel(debug=False):
    nc = bacc.Bacc(None)
    xt_d = nc.dram_tensor("xt", [DIM, N], F16, kind="ExternalInput")
    wq_d = nc.dram_tensor("wq", [DIM, 768], F16, kind="ExternalInput")
    pw_d = nc.dram_tensor("pw", [256, DIM], F16, kind="ExternalInput")
    kidx_d = nc.dram_tensor("kidx", [128, 12], I16, kind="ExternalInput")
    vkidx_d = nc.dram_tensor("vkidx", [128, 8], I16, kind="ExternalInput")
    wb_d = nc.dram_tensor("wbias", [128, 64], F32, kind="ExternalInput")
    y_d = nc.dram_tensor("y", [N, DIM], F32, kind="ExternalOutput")
    dbg = {}
    if debug:
        for nm, shp, dt in [("dbg_Kg", [128, M * 3 * BS], F16),
                            ("dbg_Vg0", [128, M * 2 * 65], F16),
                            ("dbg_vTs", [128, 16 * 2 * 65], F16),
                            ("dbg_pu", [128, 384], F16),
                            ("dbg_pts", [128, 384], F16),
                            ("dbg_rcp", [128, 2], F32),
                            ("dbg_avsb", [128, 128], F16)]:
            dbg[nm] = nc.dram_tensor(nm, shp, dt, kind="ExternalOutput")

    with tile.TileContext(nc) as tc:
        with tc.tile_pool(name="big", bufs=1) as big, \
             tc.tile_pool(name="wrk", bufs=4) as wrk:

            # ---- persistent SBUF tensors ----
            xt = big.tile([128, 8, N], F16)           # x^T, feature-major
            wq = big.tile([128, 8, 768], F16)         # qkv weights^T
            pwt = big.tile([128, 2, DIM], F16)        # proj weights
            kidx = big.tile([128, 12], I16)
            vkidx = big.tile([128, 8], I16)
            wb = big.tile([128, 64], F32)
            qT = [big.tile([128, N], F16, name=f"qT{i}") for i in range(2)]
            kkT = [big.tile([128, N], F16, name=f"kkT{i}") for i in range(2)]
            vvT = [big.tile([128, N], F16, name=f"vvT{i}") for i in range(2)]
            # v^T chunks [keys128, (hdA|1|hdB|1) x 16 chunks], ones at 64/129
            vTs = [big.tile([128, 16, 2, 65], F16, name=f"vTs{i}")
                   for i in range(2)]
            Kg = [big.tile([128, M * 3, BS], F16, name=f"Kg{i}")
                  for i in range(2)]
            # V gathered hd-major (ap_gather), then transposed into Vg:
            # [keys(s0|s1), qb, head, hd+ones]
            Vgh = [big.tile([128, M * 2, BS], F16, name=f"Vgh{i}")
                   for i in range(2)]
            Vg = [big.tile([128, M, 2, 65], F16, name=f"Vg{i}")
                  for i in range(2)]
            outT = [big.tile([128, N], F16, name=f"outT{i}") for i in range(2)]
            identf = big.tile([128, 128], F32)
            ident = big.tile([128, 128], F16)

            # ---- input DMAs (split for pipelining) ----
            xt_v = xt_d[:].rearrange("(a p) n -> p a n", p=128)
            wq_v = wq_d[:].rearrange("(a p) n -> p a n", p=128)
            pw_v = pw_d[:].rearrange("(a p) n -> p a n", p=128)
            for k in range(8):
                nc.sync.dma_start(xt[:, k, :], xt_v[:, k, :])
                nc.sync.dma_start(wq[:, k, :], wq_v[:, k, :])
            nc.sync.dma_start(pwt[:], pw_v[:])
            nc.sync.dma_start(kidx[:], kidx_d[:])
            nc.sync.dma_start(vkidx[:], vkidx_d[:])
            nc.sync.dma_start(wb[:], wb_d[:])

            make_identity(nc, identf[:])
            nc.vector.tensor_copy(ident[:], identf[:])
            for p in range(2):
                nc.gpsimd.memset(Vg[p][:, :, :, 64:65], 1.0)
                nc.gpsimd.memset(vTs[p][:, :, :, 64:65], 1.0)

            # ---- kv projection: fp16, accumulate over 8 K-chunks ----
            # (q strips are interleaved into the attention loop to keep the
            # PE array dense for the HAM clock gate)
            # col layout: 0:128 q-p0 | 128:256 q-p1 | 256:384 k-p0 |
            #             384:512 v-p0 | 512:640 k-p1 | 640:768 v-p1
            mt_order = [2, 3, 4, 5]
            tgt = {2: kkT[0], 3: vvT[0], 4: kkT[1], 5: vvT[1],
                   0: qT[0], 1: qT[1]}
            with tc.tile_pool(name="qkps", bufs=8, space="PSUM") as qkps:
                for mt in mt_order:
                    for nt in range(4):
                        ps = qkps.tile([128, 512], F32)
                        for k in range(8):
                            nc.tensor.matmul(
                                ps[:], lhsT=wq[:, k, mt * 128:(mt + 1) * 128],
                                rhs=xt[:, k, nt * 512:(nt + 1) * 512],
                                start=(k == 0), stop=(k == 7))
                        nc.vector.tensor_copy(
                            tgt[mt][:, nt * 512:(nt + 1) * 512], ps[:])

            # ---- v^T: transpose vvT to keys-major, append ones cols ----
            with tc.tile_pool(name="vtps", bufs=4, space="PSUM") as vtps:
                for p in range(2):
                    for c in range(16):
                        tp = vtps.tile([128, 128], F16)
                        nc.tensor.transpose(
                            tp[:], vvT[p][:, c * 128:(c + 1) * 128], ident[:])
                        nc.vector.tensor_copy(
                            vTs[p][:, c, :, 0:64],
                            tp[:].rearrange("p (h d) -> p h d", h=2))

                    # K gather: one ap_gather per pair (A idx in partition
                    # groups 0-3, B idx in groups 4-7)
                    nc.gpsimd.ap_gather(
                        Kg[p][:], kkT[p][:], kidx[:, p * 6:(p + 1) * 6],
                        channels=128, num_elems=M, d=BS, num_idxs=M * 3)
                    # V gather (hd-major blocks: s0, s1 per query block)
                    nc.gpsimd.ap_gather(
                        Vgh[p][:], vvT[p][:], vkidx[:, p * 4:(p + 1) * 4],
                        channels=128, num_elems=M, d=BS, num_idxs=M * 2)
                    # transpose gathered V blocks to keys-major:
                    # slot0 -> partitions 0:64, slot1 -> 64:128
                    for q2 in range(M // 2):
                        vp = vtps.tile([128, 2, 128], F16)
                        for j in range(2):
                            qb = 2 * q2 + j
                            nc.tensor.transpose(
                                vp[0:64, j, :], Vgh[p][:, 2 * qb, :],
                                ident[:])
                            nc.tensor.transpose(
                                vp[64:128, j, :], Vgh[p][:, 2 * qb + 1, :],
                                ident[:], tile_position=(0, 64))
                        if q2 % 2 == 0:
                            nc.vector.tensor_copy(
                                Vg[p][:, 2 * q2:2 * q2 + 2, :, 0:64],
                                vp[:].rearrange("p a (h d) -> p a h d", h=2))
                        else:
                            nc.scalar.copy(
                                Vg[p][:, 2 * q2:2 * q2 + 2, :, 0:64],
                                vp[:].rearrange("p a (h d) -> p a h d", h=2))

            if debug:
                nc.sync.dma_start(
                    dbg["dbg_Kg"][:], Kg[0][:].rearrange("p a b -> p (a b)"))
                nc.sync.dma_start(
                    dbg["dbg_Vg0"][:],
                    Vg[0][:].rearrange("p a b c -> p (a b c)"))
                nc.sync.dma_start(
                    dbg["dbg_vTs"][:],
                    vTs[0][:].rearrange("p a b c -> p (a b c)"))

            # ---- block-sparse attention + interleaved projections ----
            # scores computed TRANSPOSED (keys on partitions) via gathered-K
            # as the stationary operand: St[:, 0:64]=A-j0, 64:128=A-j1,
            # 128:192=B-j0, 192:256=B-j1 (keys s0|s1 on partitions);
            # cols 256:320=A-diag, 320:384=B-diag (j0 rows 0:64, j1 64:128).
            with tc.tile_pool(name="spsp", bufs=2, space="PSUM") as spsp, \
                 tc.tile_pool(name="ptps", bufs=2, space="PSUM") as ptps, \
                 tc.tile_pool(name="avps", bufs=2, space="PSUM") as avps, \
                 tc.tile_pool(name="otps", bufs=1, space="PSUM") as otps, \
                 tc.tile_pool(name="qps2", bufs=1, space="PSUM") as qps2:
                ypsp = qps2
                for qq in range(16):
                    if qq % 4 == 0:
                        # q-projection strip for the next 512 tokens: dense
                        # f512 matmuls that keep the HAM clock gate warm
                        nt = qq // 4
                        for mt in range(2):
                            ps = qps2.tile([128, 512], F32, tag="big5")
                            for k in range(8):
                                nc.tensor.matmul(
                                    ps[:],
                                    lhsT=wq[:, k, mt * 128:(mt + 1) * 128],
                                    rhs=xt[:, k, nt * 512:(nt + 1) * 512],
                                    start=(k == 0), stop=(k == 7))
                            nc.vector.tensor_copy(
                                tgt[mt][:, nt * 512:(nt + 1) * 512], ps[:])
                    for p in range(2):
                        sps = spsp.tile([128, 384], F32)
                        # scores for qb = 2qq, 2qq+1; head A rows 0:64,
                        # head B rows 64:128 (own gathered blocks)
                        for j in range(2):
                            qb = 2 * qq + j
                            qs = slice(qb * 64, (qb + 1) * 64)
                            ks = slice(qb * 3, (qb + 1) * 3)
                            cs = slice(j * 192, (j + 1) * 192)
                            nc.tensor.matmul(
                                sps[0:64, cs], lhsT=qT[p][0:64, qs],
                                rhs=Kg[p][0:64, ks, :], start=True, stop=True)
                            nc.tensor.matmul(
                                sps[64:128, cs], lhsT=qT[p][64:128, qs],
                                rhs=Kg[p][64:128, ks, :], start=True,
                                stop=True, skip_group_check=True)
                        # mask duplicated diag slot (bias -30000 -> exp 0)
                        for j in range(2):
                            qb = 2 * qq + j
                            ds_ = slice(j * 192 + 128, j * 192 + 192)
                            nc.vector.tensor_scalar(
                                sps[:, ds_], sps[:, ds_],
                                wb[:, p * 32 + qb:p * 32 + qb + 1], None,
                                op0=mybir.AluOpType.add)
                        # exp (unnormalized probs)
                        pu = wrk.tile([128, 384], F16, tag="pu")
                        nc.scalar.activation(pu[:], sps[:],
                                             mybir.ActivationFunctionType.Exp)
                        # transpose probs: slots01 [128k x 128q] per j at
                        # cols 0:128 / 128:256; diag slots share cols
                        # 256:384 (j=0 -> partitions 0:64, j=1 -> 64:128,
                        # matching vTs key-partition parity)
                        pt = ptps.tile([128, 384], F16)
                        for j in range(2):
                            nc.tensor.transpose(
                                pt[:, j * 128:j * 128 + 128],
                                pu[:, j * 192:j * 192 + 128], ident[:])
                            nc.tensor.transpose(
                                pt[j * 64:j * 64 + 64, 256:384],
                                pu[:, j * 192 + 128:j * 192 + 192], ident[:],
                                tile_position=(0, j * 64))
                        pts = wrk.tile([128, 384], F16, tag="pts")
                        nc.vector.tensor_copy(pts[:], pt[:])
                        # AV (+ ones-column -> denominator in col 64)
                        avp = avps.tile([128, 2, 65], F32)
                        for j in range(2):
                            qb = 2 * qq + j
                            dg = slice(j * 64, j * 64 + 64)
                            co = j * 128
                            nc.tensor.matmul(
                                avp[0:64, j, :], lhsT=pts[0:128, co:co + 64],
                                rhs=Vg[p][:, qb, 0, :],
                                start=True, stop=False)
                            nc.tensor.matmul(
                                avp[0:64, j, :],
                                lhsT=pts[dg, 256:320],
                                rhs=vTs[p][dg, qq, 0, :],
                                start=False, stop=True)
                            nc.tensor.matmul(
                                avp[64:128, j, :],
                                lhsT=pts[0:128, co + 64:co + 128],
                                rhs=Vg[p][:, qb, 1, :],
                                start=True, stop=False,
                                skip_group_check=True)
                            nc.tensor.matmul(
                                avp[64:128, j, :],
                                lhsT=pts[dg, 320:384],
                                rhs=vTs[p][dg, qq, 1, :],
                                start=False, stop=True,
                                skip_group_check=True)
                        # normalize while evacuating PSUM
                        rcp = wrk.tile([128, 2], F32, tag="rcp")
                        nc.vector.reciprocal(rcp[:], avp[:, :, 64])
                        av_sb = wrk.tile([128, 2, 64], F16, tag="av_sb")
                        for j in range(2):
                            nc.scalar.activation(
                                av_sb[:, j, :], avp[:, j, 0:64],
                                mybir.ActivationFunctionType.Copy,
                                scale=rcp[:, j:j + 1])
                        # transpose back to [hd, q] for the projection
                        otp = otps.tile([128, 2, 64], F16)
                        for j in range(2):
                            nc.tensor.transpose(
                                otp[0:64, j, :], av_sb[0:64, j, :],
                                ident[0:64, 0:64])
                            nc.tensor.transpose(
                                otp[64:128, j, :], av_sb[64:128, j, :],
                                ident[64:128, 64:128],
                                tile_position=(64, 64))
                        nc.vector.tensor_copy(
                            outT[p][:, qq * 128:(qq + 1) * 128],
                            otp[:].rearrange("p a b -> p (a b)"))
                        if debug and qq == 0 and p == 0:
                            nc.sync.dma_start(
                                dbg["dbg_pu"][:], pu[:])
                            nc.sync.dma_start(
                                dbg["dbg_rcp"][:], rcp[:])
                            nc.sync.dma_start(
                                dbg["dbg_avsb"][:],
                                av_sb[:].rearrange("p a b -> p (a b)"))
                    # ---- projection burst: two 128-token tiles per 2 qq
                    # (8 dense f512 matmuls to keep the clock gate warm) ----
                    if qq % 2 == 1:
                        for tt in (qq - 1, qq):
                            ts_ = slice(tt * 128, (tt + 1) * 128)
                            for nt in range(2):
                                ns = slice(nt * 512, (nt + 1) * 512)
                                yp = ypsp.tile([128, 512], F32, tag="big5")
                                nc.tensor.matmul(yp[:], lhsT=outT[0][:, ts_],
                                                 rhs=pwt[:, 0, ns],
                                                 start=True, stop=False)
                                nc.tensor.matmul(yp[:], lhsT=outT[1][:, ts_],
                                                 rhs=pwt[:, 1, ns],
                                                 start=False, stop=True)
                                ys = wrk.tile([128, 512], F32, tag="ys")
                                nc.scalar.copy(ys[:], yp[:])
                                nc.sync.dma_start(y_d[ts_, ns], ys[:])

    nc.finalize()
    return nc


def _host_prep(x, qkv_w, proj_w):
    """Per-core input maps + block selection (float64, matches fp32 ref)."""
    in_maps = []
    x64 = x.astype(np.float64)
    for core in range(NCORES):
        b = core // (NCORES // B)
        hg = core % (NCORES // B)
        heads = [hg * HPC + i for i in range(HPC)]

        xt = np.ascontiguousarray(x[b].T).astype(np.float16)

        wqkvT = np.empty((DIM, 768), np.float32)
        for p in range(2):
            hA, hB = heads[2 * p], heads[2 * p + 1]
            wqkvT[:, p*128:p*128+64] = qkv_w[hA*64:(hA+1)*64].T * SCALE
            wqkvT[:, p*128+64:p*128+128] = qkv_w[hB*64:(hB+1)*64].T * SCALE
            kbase = 256 + p * 256
            wqkvT[:, kbase:kbase+64] = qkv_w[DIM+hA*64:DIM+(hA+1)*64].T
            wqkvT[:, kbase+64:kbase+128] = qkv_w[DIM+hB*64:DIM+(hB+1)*64].T
            vbase = kbase + 128
            wqkvT[:, vbase:vbase+64] = qkv_w[2*DIM+hA*64:2*DIM+(hA+1)*64].T
            wqkvT[:, vbase+64:vbase+128] = qkv_w[2*DIM+hB*64:2*DIM+(hB+1)*64].T
        wqkvT = wqkvT.astype(np.float16)

        pw = np.ascontiguousarray(
            proj_w[:, heads[0]*64:(heads[-1]+1)*64].T).astype(np.float16)

        # float64 selection (matches fp32 reference ordering w/ margin)
        xb = x64[b].reshape(M, BS, DIM).mean(axis=1)
        kidx = np.zeros((128, 12), np.int16)
        vkidx = np.zeros((128, 8), np.int16)
        wbias = np.zeros((128, 64), np.float32)
        for p in range(2):
            for hip in range(2):
                h = heads[2 * p + hip]
                qb_ = xb @ qkv_w[h*64:(h+1)*64].T.astype(np.float64)
                kb_ = xb @ qkv_w[DIM+h*64:DIM+(h+1)*64].T.astype(np.float64)
                c = qb_ @ kb_.T
                blocks = np.empty(M * 3, np.int16)
                vblocks = np.empty(M * 2, np.int16)
                for i in range(M):
                    order = np.argsort(-c[i], kind="stable")
                    i1, i2 = int(order[0]), int(order[1])
                    blocks[i * 3:(i + 1) * 3] = (i1, i2, i)
                    vblocks[i * 2:(i + 1) * 2] = (i1, i2)
                    if i == i1 or i == i2:
                        wbias[hip*64:(hip+1)*64, p*32+i] = -30000.0
                # wrapped idx lists in each 16-partition group of this head
                wrapped = blocks.reshape(6, 16).T    # [16, 6]
                vwrapped = vblocks.reshape(4, 16).T  # [16, 4]
                for g in range(4):
                    rows = slice((hip*4+g)*16, (hip*4+g+1)*16)
                    kidx[rows, p*6:(p+1)*6] = wrapped
                    vkidx[rows, p*4:(p+1)*4] = vwrapped
        in_maps.append({"xt": xt, "wq": wqkvT, "pw": pw, "kidx": kidx,
                        "vkidx": vkidx, "wbias": wbias})
    return in_maps


def kernel(x, qkv_w, proj_w, proj_b):
    global _NC_CACHE, LAST_RESULTS
    x = np.asarray(x, np.float32)
    qkv_w = np.asarray(qkv_w, np.float32)
    proj_w = np.asarray(proj_w, np.float32)
    proj_b = np.asarray(proj_b, np.float32)

    if _NC_CACHE is None:
        _NC_CACHE = build_kernel()
    nc = _NC_CACHE

    in_maps = _host_prep(x, qkv_w, proj_w)
    res = run_bass_kernel_spmd(nc, in_maps, list(range(NCORES)))
    LAST_RESULTS = res

    out = np.zeros((B, N, DIM), np.float32)
    for core in range(NCORES):
        out[core // (NCORES // B)] += res.results[core]["y"]
    out += proj_b[None, None, :]
    return out


# revision 7
# speedup vs baseline: 1.0554x; 1.0554x over previous
"""Block-sparse attention (CABAttention) Trainium2 kernel, v2.

Sharding: 8 cores = 2 batches x 4 head-groups (4 heads each, as 2 pairs).
Key ideas vs v1:
- All dynamic block selection moved OFF the PE: K blocks gathered with a
  single gpsimd ap_gather per pair (per-16-partition-group index lists
  carry head A vs head B selections); V blocks gathered keys-major with
  indirect DMAs from a DRAM round-trip copy. Every matmul AP is static.
- Softmax denominator comes free from the AV matmul via a ones-column
  appended to the gathered V (no activation accumulator, no reduce).
  Normalization folds into the PSUM->SBUF copy of the AV output.
- fp16 inputs (x^T, qkv w, proj w) halve the input DMA.
- Output projection interleaved into the attention loop (one 128-token
  tile per 4 attention iterations) so its DMA/copies overlap.
Block selection (top-2 + diag) is computed on host in float64 and passed
as gather indices; duplicated-diag slots are masked with a -30000 bias
before exp (baseline-proven).
Unshard: partial output projections summed over the 4 cores per batch,
plus bias.
"""
import sys

sys.path.insert(0, "/opt/trn_rl_repo")

import numpy as np

import concourse.bass as bass
import concourse.mybir as mybir
import concourse.tile as tile
from concourse import bacc
from concourse.bass_utils import run_bass_kernel_spmd
from concourse.masks import make_identity

F32 = mybir.dt.float32
F16 = mybir.dt.float16
I16 = mybir.dt.int16
I32 = mybir.dt.int32

DIM = 1024
H = 16
HD = 64
BS = 64
N = 2048
B = 2
M = N // BS            # 32 blocks
SCALE = HD ** -0.5
NCORES = 8
HPC = H // (NCORES // B)   # 4 heads per core

_NC_CACHE = None
LAST_RESULTS = None


def build_kernel(debug=False):
    nc = bacc.Bacc(None)
    xt_d = nc.dram_tensor("xt", [DIM, N], F16, kind="ExternalInput")
    wq_d = nc.dram_tensor("wq", [DIM, 768], F16, kind="ExternalInput")
    pw_d = nc.dram_tensor("pw", [256, DIM], F16, kind="ExternalInput")
    kidx_d = nc.dram_tensor("kidx", [128, 12], I16, kind="ExternalInput")
    vkidx_d = nc.dram_tensor("vkidx", [128, 8], I16, kind="ExternalInput")
    wb_d = nc.dram_tensor("wbias", [128, 64], F32, kind="ExternalInput")
    y_d = nc.dram_tensor("y", [N, DIM], F32, kind="ExternalOutput")
    dbg = {}
    if debug:
        for nm, shp, dt in [("dbg_Kg", [128, M * 3 * BS], F16),
                            ("dbg_Vg0", [128, M * 2 * 65], F16),
                            ("dbg_vTs", [128, 16 * 2 * 65], F16),
                            ("dbg_pu", [128, 384], F16),
                            ("dbg_pts", [128, 384], F16),
                            ("dbg_rcp", [128, 2], F32),
                            ("dbg_avsb", [128, 128], F16)]:
            dbg[nm] = nc.dram_tensor(nm, shp, dt, kind="ExternalOutput")

    with tile.TileContext(nc) as tc:
        with tc.tile_pool(name="big", bufs=1) as big, \
             tc.tile_pool(name="wrk", bufs=4) as wrk:

            # ---- persistent SBUF tensors ----
            xt = big.tile([128, 8, N], F16)           # x^T, feature-major
            wq = big.tile([128, 8, 768], F16)         # qkv weights^T
            pwt = big.tile([128, 2, DIM], F16)        # proj weights
            kidx = big.tile([128, 12], I16)
            vkidx = big.tile([128, 8], I16)
            wb = big.tile([128, 64], F32)
            qT = [big.tile([128, N], F16, name=f"qT{i}") for i in range(2)]
            kkT = [big.tile([128, N], F16, name=f"kkT{i}") for i in range(2)]
            vvT = [big.tile([128, N], F16, name=f"vvT{i}") for i in range(2)]
            # v^T chunks [keys128, (hdA|1|hdB|1) x 16 chunks], ones at 64/129
            vTs = [big.tile([128, 16, 2, 65], F16, name=f"vTs{i}")
                   for i in range(2)]
            Kg = [big.tile([128, M * 3, BS], F16, name=f"Kg{i}")
                  for i in range(2)]
            # V gathered hd-major (ap_gather), then transposed into Vg:
            # [keys(s0|s1), qb, head, hd+ones]
            Vgh = [big.tile([128, M * 2, BS], F16, name=f"Vgh{i}")
                   for i in range(2)]
            Vg = [big.tile([128, M, 2, 65], F16, name=f"Vg{i}")
                  for i in range(2)]
            outT = [big.tile([128, N], F16, name=f"outT{i}") for i in range(2)]
            identf = big.tile([128, 128], F32)
            ident = big.tile([128, 128], F16)

            # ---- input DMAs (split for pipelining) ----
            xt_v = xt_d[:].rearrange("(a p) n -> p a n", p=128)
            wq_v = wq_d[:].rearrange("(a p) n -> p a n", p=128)
            pw_v = pw_d[:].rearrange("(a p) n -> p a n", p=128)
            for k in range(8):
                nc.sync.dma_start(xt[:, k, :], xt_v[:, k, :])
                nc.sync.dma_start(wq[:, k, :], wq_v[:, k, :])
            nc.sync.dma_start(pwt[:], pw_v[:])
            nc.sync.dma_start(kidx[:], kidx_d[:])
            nc.sync.dma_start(vkidx[:], vkidx_d[:])
            nc.sync.dma_start(wb[:], wb_d[:])

            make_identity(nc, identf[:])
            nc.vector.tensor_copy(ident[:], identf[:])
            for p in range(2):
                nc.gpsimd.memset(Vg[p][:, :, :, 64:65], 1.0)
                nc.gpsimd.memset(vTs[p][:, :, :, 64:65], 1.0)

            # ---- qkv projection: fp16, accumulate over 8 K-chunks ----
            # col layout: 0:128 q-p0 | 128:256 q-p1 | 256:384 k-p0 |
            #             384:512 v-p0 | 512:640 k-p1 | 640:768 v-p1
            mt_order = [2, 3, 4, 5, 0, 1]
            tgt = {2: kkT[0], 3: vvT[0], 4: kkT[1], 5: vvT[1],
                   0: qT[0], 1: qT[1]}
            with tc.tile_pool(name="qkps", bufs=8, space="PSUM") as qkps:
                for mt in mt_order:
                    for nt in range(4):
                        ps = qkps.tile([128, 512], F32)
                        for k in range(8):
                            nc.tensor.matmul(
                                ps[:], lhsT=wq[:, k, mt * 128:(mt + 1) * 128],
                                rhs=xt[:, k, nt * 512:(nt + 1) * 512],
                                start=(k == 0), stop=(k == 7))
                        nc.vector.tensor_copy(
                            tgt[mt][:, nt * 512:(nt + 1) * 512], ps[:])

            # ---- v^T: transpose vvT to keys-major, append ones cols ----
            with tc.tile_pool(name="vtps", bufs=4, space="PSUM") as vtps:
                for p in range(2):
                    for c in range(16):
                        tp = vtps.tile([128, 128], F16)
                        nc.tensor.transpose(
                            tp[:], vvT[p][:, c * 128:(c + 1) * 128], ident[:])
                        nc.vector.tensor_copy(
                            vTs[p][:, c, :, 0:64],
                            tp[:].rearrange("p (h d) -> p h d", h=2))

                    # K gather: one ap_gather per pair (A idx in partition
                    # groups 0-3, B idx in groups 4-7)
                    nc.gpsimd.ap_gather(
                        Kg[p][:], kkT[p][:], kidx[:, p * 6:(p + 1) * 6],
                        channels=128, num_elems=M, d=BS, num_idxs=M * 3)
                    # V gather (hd-major blocks: s0, s1 per query block)
                    nc.gpsimd.ap_gather(
                        Vgh[p][:], vvT[p][:], vkidx[:, p * 4:(p + 1) * 4],
                        channels=128, num_elems=M, d=BS, num_idxs=M * 2)
                    # transpose gathered V blocks to keys-major:
                    # slot0 -> partitions 0:64, slot1 -> 64:128
                    for q2 in range(M // 2):
                        vp = vtps.tile([128, 2, 128], F16)
                        for j in range(2):
                            qb = 2 * q2 + j
                            nc.tensor.transpose(
                                vp[0:64, j, :], Vgh[p][:, 2 * qb, :],
                                ident[:])
                            nc.tensor.transpose(
                                vp[64:128, j, :], Vgh[p][:, 2 * qb + 1, :],
                                ident[:], tile_position=(0, 64))
                        if q2 % 2 == 0:
                            nc.vector.tensor_copy(
                                Vg[p][:, 2 * q2:2 * q2 + 2, :, 0:64],
                                vp[:].rearrange("p a (h d) -> p a h d", h=2))
                        else:
                            nc.scalar.copy(
                                Vg[p][:, 2 * q2:2 * q2 + 2, :, 0:64],
                                vp[:].rearrange("p a (h d) -> p a h d", h=2))

            if debug:
                nc.sync.dma_start(
                    dbg["dbg_Kg"][:], Kg[0][:].rearrange("p a b -> p (a b)"))
                nc.sync.dma_start(
                    dbg["dbg_Vg0"][:],
                    Vg[0][:].rearrange("p a b c -> p (a b c)"))
                nc.sync.dma_start(
                    dbg["dbg_vTs"][:],
                    vTs[0][:].rearrange("p a b c -> p (a b c)"))

            # ---- block-sparse attention + interleaved projections ----
            # scores computed TRANSPOSED (keys on partitions) via gathered-K
            # as the stationary operand: St[:, 0:64]=A-j0, 64:128=A-j1,
            # 128:192=B-j0, 192:256=B-j1 (keys s0|s1 on partitions);
            # cols 256:320=A-diag, 320:384=B-diag (j0 rows 0:64, j1 64:128).
            with tc.tile_pool(name="spsp", bufs=2, space="PSUM") as spsp, \
                 tc.tile_pool(name="ptps", bufs=2, space="PSUM") as ptps, \
                 tc.tile_pool(name="avps", bufs=2, space="PSUM") as avps, \
                 tc.tile_pool(name="otps", bufs=1, space="PSUM") as otps, \
                 tc.tile_pool(name="ypsp", bufs=1, space="PSUM") as ypsp:
                for qq in range(16):
                    for p in range(2):
                        sps = spsp.tile([128, 384], F32)
                        # scores for qb = 2qq, 2qq+1; head A rows 0:64,
                        # head B rows 64:128 (own gathered blocks)
                        for j in range(2):
                            qb = 2 * qq + j
                            qs = slice(qb * 64, (qb + 1) * 64)
                            ks = slice(qb * 3, (qb + 1) * 3)
                            cs = slice(j * 192, (j + 1) * 192)
                            nc.tensor.matmul(
                                sps[0:64, cs], lhsT=qT[p][0:64, qs],
                                rhs=Kg[p][0:64, ks, :], start=True, stop=True)
                            nc.tensor.matmul(
                                sps[64:128, cs], lhsT=qT[p][64:128, qs],
                                rhs=Kg[p][64:128, ks, :], start=True,
                                stop=True, skip_group_check=True)
                        # mask duplicated diag slot (bias -30000 -> exp 0)
                        for j in range(2):
                            qb = 2 * qq + j
                            ds_ = slice(j * 192 + 128, j * 192 + 192)
                            nc.vector.tensor_scalar(
                                sps[:, ds_], sps[:, ds_],
                                wb[:, p * 32 + qb:p * 32 + qb + 1], None,
                                op0=mybir.AluOpType.add)
                        # exp (unnormalized probs)
                        pu = wrk.tile([128, 384], F16, tag="pu")
                        nc.scalar.activation(pu[:], sps[:],
                                             mybir.ActivationFunctionType.Exp)
                        # transpose probs: slots01 [128k x 128q] per j at
                        # cols 0:128 / 128:256; diag slots share cols
                        # 256:384 (j=0 -> partitions 0:64, j=1 -> 64:128,
                        # matching vTs key-partition parity)
                        pt = ptps.tile([128, 384], F16)
                        for j in range(2):
                            nc.tensor.transpose(
                                pt[:, j * 128:j * 128 + 128],
                                pu[:, j * 192:j * 192 + 128], ident[:])
                            nc.tensor.transpose(
                                pt[j * 64:j * 64 + 64, 256:384],
                                pu[:, j * 192 + 128:j * 192 + 192], ident[:],
                                tile_position=(0, j * 64))
                        pts = wrk.tile([128, 384], F16, tag="pts")
                        nc.vector.tensor_copy(pts[:], pt[:])
                        # AV (+ ones-column -> denominator in col 64)
                        avp = avps.tile([128, 2, 65], F32)
                        for j in range(2):
                            qb = 2 * qq + j
                            dg = slice(j * 64, j * 64 + 64)
                            co = j * 128
                            nc.tensor.matmul(
                                avp[0:64, j, :], lhsT=pts[0:128, co:co + 64],
                                rhs=Vg[p][:, qb, 0, :],
                                start=True, stop=False)
                            nc.tensor.matmul(
                                avp[0:64, j, :],
                                lhsT=pts[dg, 256:320],
                                rhs=vTs[p][dg, qq, 0, :],
                                start=False, stop=True)
                            nc.tensor.matmul(
                                avp[64:128, j, :],
                                lhsT=pts[0:128, co + 64:co + 128],
                                rhs=Vg[p][:, qb, 1, :],
                                start=True, stop=False,
                                skip_group_check=True)
                            nc.tensor.matmul(
                                avp[64:128, j, :],
                                lhsT=pts[dg, 320:384],
                                rhs=vTs[p][dg, qq, 1, :],
                                start=False, stop=True,
                                skip_group_check=True)
                        # normalize while evacuating PSUM
                        rcp = wrk.tile([128, 2], F32, tag="rcp")
                        nc.vector.reciprocal(rcp[:], avp[:, :, 64])
                        av_sb = wrk.tile([128, 2, 64], F16, tag="av_sb")
                        for j in range(2):
                            nc.scalar.activation(
                                av_sb[:, j, :], avp[:, j, 0:64],
                                mybir.ActivationFunctionType.Copy,
                                scale=rcp[:, j:j + 1])
                        # transpose back to [hd, q] for the projection
                        otp = otps.tile([128, 2, 64], F16)
                        for j in range(2):
                            nc.tensor.transpose(
                                otp[0:64, j, :], av_sb[0:64, j, :],
                                ident[0:64, 0:64])
                            nc.tensor.transpose(
                                otp[64:128, j, :], av_sb[64:128, j, :],
                                ident[64:128, 64:128],
                                tile_position=(64, 64))
                        nc.vector.tensor_copy(
                            outT[p][:, qq * 128:(qq + 1) * 128],
                            otp[:].rearrange("p a b -> p (a b)"))
                        if debug and qq == 0 and p == 0:
                            nc.sync.dma_start(
                                dbg["dbg_pu"][:], pu[:])
                            nc.sync.dma_start(
                                dbg["dbg_rcp"][:], rcp[:])
                            nc.sync.dma_start(
                                dbg["dbg_avsb"][:],
                                av_sb[:].rearrange("p a b -> p (a b)"))
                    # ---- projection for this 128-token tile ----
                    ts_ = slice(qq * 128, (qq + 1) * 128)
                    for nt in range(2):
                        ns = slice(nt * 512, (nt + 1) * 512)
                        yp = ypsp.tile([128, 512], F32)
                        nc.tensor.matmul(yp[:], lhsT=outT[0][:, ts_],
                                         rhs=pwt[:, 0, ns],
                                         start=True, stop=False)
                        nc.tensor.matmul(yp[:], lhsT=outT[1][:, ts_],
                                         rhs=pwt[:, 1, ns],
                                         start=False, stop=True)
                        ys = wrk.tile([128, 512], F32, tag="ys")
                        nc.scalar.copy(ys[:], yp[:])
                        nc.sync.dma_start(y_d[ts_, ns], ys[:])

    nc.finalize()
    return nc


def _host_prep(x, qkv_w, proj_w):
    """Per-core input maps + block selection (float64, matches fp32 ref)."""
    in_maps = []
    x64 = x.astype(np.float64)
    for core in range(NCORES):
        b = core // (NCORES // B)
        hg = core % (NCORES // B)
        heads = [hg * HPC + i for i in range(HPC)]

        xt = np.ascontiguousarray(x[b].T).astype(np.float16)

        wqkvT = np.empty((DIM, 768), np.float32)
        for p in range(2):
            hA, hB = heads[2 * p], heads[2 * p + 1]
            wqkvT[:, p*128:p*128+64] = qkv_w[hA*64:(hA+1)*64].T * SCALE
            wqkvT[:, p*128+64:p*128+128] = qkv_w[hB*64:(hB+1)*64].T * SCALE
            kbase = 256 + p * 256
            wqkvT[:, kbase:kbase+64] = qkv_w[DIM+hA*64:DIM+(hA+1)*64].T
            wqkvT[:, kbase+64:kbase+128] = qkv_w[DIM+hB*64:DIM+(hB+1)*64].T
            vbase = kbase + 128
            wqkvT[:, vbase:vbase+64] = qkv_w[2*DIM+hA*64:2*DIM+(hA+1)*64].T
            wqkvT[:, vbase+64:vbase+128] = qkv_w[2*DIM+hB*64:2*DIM+(hB+1)*64].T
        wqkvT = wqkvT.astype(np.float16)

        pw = np.ascontiguousarray(
            proj_w[:, heads[0]*64:(heads[-1]+1)*64].T).astype(np.float16)

        # float64 selection (matches fp32 reference ordering w/ margin)
        xb = x64[b].reshape(M, BS, DIM).mean(axis=1)
        kidx = np.zeros((128, 12), np.int16)
        vkidx = np.zeros((128, 8), np.int16)
        wbias = np.zeros((128, 64), np.float32)
        for p in range(2):
            for hip in range(2):
                h = heads[2 * p + hip]
                qb_ = xb @ qkv_w[h*64:(h+1)*64].T.astype(np.float64)
                kb_ = xb @ qkv_w[DIM+h*64:DIM+(h+1)*64].T.astype(np.float64)
                c = qb_ @ kb_.T
                blocks = np.empty(M * 3, np.int16)
                vblocks = np.empty(M * 2, np.int16)
                for i in range(M):
                    order = np.argsort(-c[i], kind="stable")
                    i1, i2 = int(order[0]), int(order[1])
                    blocks[i * 3:(i + 1) * 3] = (i1, i2, i)
                    vblocks[i * 2:(i + 1) * 2] = (i1, i2)
                    if i == i1 or i == i2:
                        wbias[hip*64:(hip+1)*64, p*32+i] = -30000.0
                # wrapped idx lists in each 16-partition group of this head
                wrapped = blocks.reshape(6, 16).T    # [16, 6]
                vwrapped = vblocks.reshape(4, 16).T  # [16, 4]
                for g in range(4):
                    rows = slice((hip*4+g)*16, (hip*4+g+1)*16)
                    kidx[rows, p*6:(p+1)*6] = wrapped
                    vkidx[rows, p*4:(p+1)*4] = vwrapped
        in_maps.append({"xt": xt, "wq": wqkvT, "pw": pw, "kidx": kidx,
                        "vkidx": vkidx, "wbias": wbias})
    return in_maps


def kernel(x, qkv_w, proj_w, proj_b):
    global _NC_CACHE, LAST_RESULTS
    x = np.asarray(x, np.float32)
    qkv_w = np.asarray(qkv_w, np.float32)
    proj_w = np.asarray(proj_w, np.float32)
    proj_b = np.asarray(proj_b, np.float32)

    if _NC_CACHE is None:
        _NC_CACHE = build_kernel()
    nc = _NC_CACHE

    in_maps = _host_prep(x, qkv_w, proj_w)
    res = run_bass_kernel_spmd(nc, in_maps, list(range(NCORES)))
    LAST_RESULTS = res

    out = np.zeros((B, N, DIM), np.float32)
    for core in range(NCORES):
        out[core // (NCORES // B)] += res.results[core]["y"]
    out += proj_b[None, None, :]
    return out
